# revision 1
# baseline (speedup 1.0000x reference)
"""GCN message-passing layer (GCNConv + skip + BatchNorm + ReLU) on 8 TRN2 cores.

Strategy (matches the "shard nodes / partition edges by target" hint):
  - Nodes sharded across 8 cores (12500 each, padded to 12544 = 98*128).
  - Edges (incl. self-loops) partitioned by target-node owner, grouped by
    target 128-node tile and by source bank (host-side index preprocessing).
  - Aggregation commutes with the linear layer: aggregate in 64-dim space,
    then one matmul. Per-node normalization dinv[c]*sum(dinv[r]*x[r]) with
    y = dinv*x computed on device, stored bf16 hi/lo (f32-accurate),
    AllGathered so every core can gather any source row locally.
  - Per 128-edge chunk: dma_gather (Q7 SWDGE ucode, int16 indices into
    <=32k-row banks) of y rows -> one-hot selection matrix S (DVE is_equal
    vs iota) -> PE matmul S^T @ [y_hi|y_lo] accumulated in PSUM per node
    tile = segment sum.
  - out = dinv*agg @ W + x @ skip_W  (bias dropped: BatchNorm cancels it),
    BN batch stats via cross-core AllReduce, BN + ReLU applied on device.
"""

import numpy as np
import ml_dtypes

P = 128
BANK_MAX = 32768

_BF16 = ml_dtypes.bfloat16

_KCACHE = {}


def _host_prep(x, edge_index, W, skip_W, gamma, beta, M, IN, OUT, GT):
    """Pure index/layout preprocessing + sharding. All float math on x stays
    on device; here we only partition/sort edges, count degrees and lay out
    per-core arrays."""
    N = x.shape[0]
    SH = N // M
    T = -(-SH // P)
    SHP = T * P
    NFP = M * SHP
    NB = -(-NFP // BANK_MAX)
    BK = NFP // NB
    assert NFP % NB == 0 and BK <= BANK_MAX
    assert T % GT == 0

    row = edge_index[0].astype(np.int64)
    col = edge_index[1].astype(np.int64)
    loops = np.arange(N, dtype=np.int64)
    row_f = np.concatenate([row, loops])
    col_f = np.concatenate([col, loops])
    E2 = row_f.shape[0]

    deg = np.bincount(col_f, minlength=N).astype(np.float32)  # >=1 (self loops)

    # Degree-balanced node->(tile,slot) assignment per core: snake round-robin
    # over tiles by descending degree equalizes per-tile edge counts, which
    # minimizes the uniform per-(tile,bank) chunk count Cb (padded gather
    # descriptors are pure Q7 desc-gen waste). node_pos[n] = padded in-core
    # position (tile*128 + slot) of global node n; also used for the source
    # table layout so y rows live at permuted positions.
    node_pos = np.empty(N, dtype=np.int64)
    for m in range(M):
        dg = deg[m * SH:(m + 1) * SH]
        order_n = np.argsort(-dg, kind="stable")
        ranks = np.empty(SH, dtype=np.int64)
        ranks[order_n] = np.arange(SH)
        rounds = ranks // T
        tpos = ranks % T
        tile_of = np.where(rounds % 2 == 0, tpos, T - 1 - tpos)
        slot_of = rounds
        node_pos[m * SH:(m + 1) * SH] = tile_of * P + slot_of

    # padded-global source row inside the AllGathered (per-core padded) table
    src_pad_all = (row_f // SH) * SHP + node_pos[row_f]
    bank_all = src_pad_all // BK

    core_all = col_f // SH
    pos_t = node_pos[col_f]
    tile_all = core_all * T + pos_t // P                     # (core,tile) id
    grp_all = tile_all * NB + bank_all                       # (core,tile,bank)

    order = np.argsort(grp_all, kind="stable")
    grp_s = grp_all[order]
    src_s = (src_pad_all - bank_all * BK)[order].astype(np.int64)  # in-bank row
    col_loc = pos_t[order] % P

    NGRP = M * T * NB
    cnts = np.bincount(grp_s, minlength=NGRP)
    Cb = max(1, int(-(-cnts.max() // P)))
    EPG = Cb * P

    starts = np.zeros(NGRP + 1, dtype=np.int64)
    np.cumsum(cnts, out=starts[1:])
    pos = np.arange(E2, dtype=np.int64) - starts[grp_s]

    gidx = np.zeros((NGRP, EPG), dtype=np.int16)
    colx = np.full((NGRP, EPG), -1.0, dtype=np.float32)
    flat = grp_s * EPG + pos
    gidx.reshape(-1)[flat] = src_s.astype(np.int16)
    colx.reshape(-1)[flat] = col_loc.astype(np.float32)

    Q = GT * Cb * P             # indices per gather call
    NCALL = (T // GT) * NB      # gather calls per core

    in_maps = []
    for m in range(M):
        pos_m = node_pos[m * SH:(m + 1) * SH]
        x_own = np.zeros((SHP, IN), dtype=np.float32)
        x_own[pos_m] = x[m * SH:(m + 1) * SH]
        deg_own = np.ones(SHP, dtype=np.float32)
        deg_own[pos_m] = deg[m * SH:(m + 1) * SH]
        mask_own = np.zeros(SHP, dtype=np.float32)
        mask_own[pos_m] = 1.0

        # per-core [T, NB, Cb*P] views
        g_m = gidx[m * T * NB:(m + 1) * T * NB].reshape(T, NB, EPG)
        c_m = colx[m * T * NB:(m + 1) * T * NB].reshape(T, NB, EPG)

        # gather-call index blocks, wrapped for the Q7 ucode:
        # call (g, b) covers tiles [g*GT,(g+1)*GT) bank b, flat order
        # (tt, k, p); wrapped = flat.reshape(Q//16,16).T tiled to 128 rows.
        blocks = []
        for g in range(T // GT):
            for b in range(NB):
                fl = g_m[g * GT:(g + 1) * GT, b].reshape(Q)
                blocks.append(np.tile(fl.reshape(Q // 16, 16).T, (8, 1)))
        gidx_w = np.concatenate(blocks, axis=1)  # [128, NCALL*Q//16]

        # colx sbuf layout: column (t, b, k) = t*NB*Cb + b*Cb + k
        c_sb = c_m.reshape(T * NB * Cb, P).T

        in_maps.append({
            "xtl": np.ascontiguousarray(
                x_own.reshape(T, P, IN).transpose(1, 0, 2).reshape(P, T * IN)),
            "xT": np.ascontiguousarray(x_own.T),
            "deg": np.ascontiguousarray(deg_own.reshape(T, P).T),
            "mask": np.ascontiguousarray(mask_own.reshape(T, P).T),
            "gidx": np.ascontiguousarray(gidx_w),
            "colx": np.ascontiguousarray(c_sb),
            "iota": np.ascontiguousarray(
                np.tile(np.arange(P, dtype=np.float32), (P, 1)).astype(_BF16)),
            "W": np.ascontiguousarray(W.astype(np.float32)),
            "skipW": np.ascontiguousarray(skip_W.astype(np.float32)),
            "gamma": np.ascontiguousarray(gamma.astype(np.float32).reshape(1, OUT)),
            "beta": np.ascontiguousarray(beta.astype(np.float32).reshape(1, OUT)),
        })
    return in_maps, Cb, NB, SH, T, SHP, node_pos


def _build(M, N, IN, OUT, T, Cb, NB, GT, debug_stop="full"):
    """Build the Bass/Tile kernel. GT = node tiles per gather call group.
    debug_stop: "A" = y-build+AllGather only; "B" = + gathers (no matmuls);
    "C" = + segment-sum main loop, v written raw (no BN collective);
    "full" = everything."""
    from concourse import bacc, mybir, tile, library_config
    from concourse.masks import make_identity

    dt = mybir.dt
    Alu = mybir.AluOpType
    Act = mybir.ActivationFunctionType

    SHP = T * P
    NFP = M * SHP
    BK = NFP // NB
    IN2 = 2 * IN            # bf16 hi|lo row width
    BN_EPS = 1e-5
    Q = GT * Cb * P
    NG = T // GT

    nc = bacc.Bacc("TRN2", target_bir_lowering=False, debug=False,
                   num_devices=M)

    xtl_d = nc.dram_tensor("xtl", [P, T * IN], dt.float32, kind="ExternalInput")
    xT_d = nc.dram_tensor("xT", [IN, SHP], dt.float32, kind="ExternalInput")
    deg_d = nc.dram_tensor("deg", [P, T], dt.float32, kind="ExternalInput")
    mask_d = nc.dram_tensor("mask", [P, T], dt.float32, kind="ExternalInput")
    gidx_d = nc.dram_tensor("gidx", [P, NG * NB * (Q // 16)], dt.int16,
                            kind="ExternalInput")
    colx_d = nc.dram_tensor("colx", [P, T * NB * Cb], dt.float32,
                            kind="ExternalInput")
    iota_d = nc.dram_tensor("iota", [P, P], dt.bfloat16, kind="ExternalInput")
    W_d = nc.dram_tensor("W", [IN, OUT], dt.float32, kind="ExternalInput")
    skipW_d = nc.dram_tensor("skipW", [IN, OUT], dt.float32, kind="ExternalInput")
    gamma_d = nc.dram_tensor("gamma", [1, OUT], dt.float32, kind="ExternalInput")
    beta_d = nc.dram_tensor("beta", [1, OUT], dt.float32, kind="ExternalInput")
    out_d = nc.dram_tensor("out", [SHP, OUT], dt.float32, kind="ExternalOutput")

    y_local = nc.dram_tensor("y_local", [SHP, IN2], dt.bfloat16)
    y_full = nc.dram_tensor("y_full", [NFP, IN2], dt.bfloat16)
    st_local = nc.dram_tensor("st_local", [1, 2 * OUT], dt.float32)
    st_global = nc.dram_tensor("st_global", [1, 2 * OUT], dt.float32,
                               addr_space="Shared")

    rg = [list(range(M))]

    with tile.TileContext(nc) as tc:
        with (
            tc.tile_pool(name="const", bufs=1) as cpool,
            tc.tile_pool(name="xload", bufs=3) as xpool,
            tc.tile_pool(name="ybuild", bufs=3) as ypool,
            tc.tile_pool(name="gather", bufs=2) as gpool,
            tc.tile_pool(name="gidxp", bufs=2) as gxpool,
            tc.tile_pool(name="sel", bufs=4) as spool,
            tc.tile_pool(name="evac", bufs=3) as epool,
            tc.tile_pool(name="outt", bufs=3) as opool,
            tc.tile_pool(name="ps_agg", bufs=2, space="PSUM") as ps_agg,
            tc.tile_pool(name="ps_tr", bufs=2, space="PSUM") as ps_tr,
            tc.tile_pool(name="ps_out", bufs=2, space="PSUM") as ps_out,
        ):
            # GPSIMD ucode library loads are inserted automatically by
            # Bacc.insert_library_loads() at compile time.

            # ---- constants / persistent state ----
            W_sb = cpool.tile([IN, OUT], dt.float32, tag="W")
            nc.sync.dma_start(W_sb[:], W_d[:, :])
            skipW_sb = cpool.tile([IN, OUT], dt.float32, tag="skipW")
            nc.sync.dma_start(skipW_sb[:], skipW_d[:, :])
            iota_sb = cpool.tile([P, P], dt.bfloat16, tag="iota")
            nc.sync.dma_start(iota_sb[:], iota_d[:, :])
            deg_sb = cpool.tile([P, T], dt.float32, tag="deg")
            nc.sync.dma_start(deg_sb[:], deg_d[:, :])
            mask_sb = cpool.tile([P, T], dt.float32, tag="mask")
            nc.sync.dma_start(mask_sb[:], mask_d[:, :])
            colx_sb = cpool.tile([P, T * NB * Cb], dt.float32, tag="colx")
            nc.sync.dma_start(colx_sb[:], colx_d[:, :])
            xT_sb = cpool.tile([IN, SHP], dt.float32, tag="xT")
            nc.sync.dma_start(xT_sb[:], xT_d[:, :])
            gamma_sb = cpool.tile([1, OUT], dt.float32, tag="gamma")
            nc.sync.dma_start(gamma_sb[:], gamma_d[:, :])
            beta_sb = cpool.tile([1, OUT], dt.float32, tag="beta")
            nc.sync.dma_start(beta_sb[:], beta_d[:, :])

            ident = cpool.tile([P, P], dt.float32, tag="ident")
            make_identity(nc, ident[:])
            ones_col = cpool.tile([P, 1], dt.float32, tag="ones_col")
            nc.vector.memset(ones_col[:], 1.0)
            ones_row = cpool.tile([1, P], dt.float32, tag="ones_row")
            nc.vector.memset(ones_row[:], 1.0)

            vbuf = cpool.tile([P, T * OUT], dt.float32, tag="vbuf")
            acc_sum = cpool.tile([P, OUT], dt.float32, tag="acc_sum")
            acc_sq = cpool.tile([P, OUT], dt.float32, tag="acc_sq")

            # dinv = sqrt(1/deg)   (ACT Rsqrt is banned for accuracy)
            dinv_sb = cpool.tile([P, T], dt.float32, tag="dinv")
            rec_t = cpool.tile([P, T], dt.float32, tag="rec_t")
            nc.vector.reciprocal(rec_t[:], deg_sb[:])
            nc.scalar.activation(dinv_sb[:], rec_t[:], Act.Sqrt)

            # ---- phase A: y = dinv * x, bf16 hi/lo, AllGather ----
            for t in range(T):
                xt_ = xpool.tile([P, IN], dt.float32, tag="xt_")
                nc.sync.dma_start(xt_[:], xtl_d[:, t * IN:(t + 1) * IN])
                y32 = xpool.tile([P, IN], dt.float32, tag="y32")
                nc.vector.tensor_scalar(
                    y32[:], xt_[:], dinv_sb[:, t:t + 1], None, Alu.mult)
                ypk = ypool.tile([P, IN2], dt.bfloat16, tag="ypk")
                nc.vector.tensor_copy(ypk[:, 0:IN], y32[:])
                nc.vector.tensor_tensor(
                    ypk[:, IN:IN2], y32[:], ypk[:, 0:IN], Alu.subtract)
                nc.sync.dma_start(y_local[t * P:(t + 1) * P, :], ypk[:])

            nc.gpsimd.collective_compute(
                "AllGather", Alu.bypass, replica_groups=rg,
                ins=[y_local.ap().opt()], outs=[y_full.ap().opt()])

            if debug_stop == "A":
                # read back a y_full slab so the AllGather result is checkable
                chk = opool.tile([P, IN2], dt.bfloat16, tag="o1")
                nc.sync.dma_start(chk[:], y_full[0:P, :])
                o2 = opool.tile([P, OUT], dt.float32, tag="o2")
                nc.vector.memset(o2[:], 0.0)
                nc.vector.tensor_copy(o2[:, 0:IN2], chk[:])
                for t in range(T):
                    nc.sync.dma_start(out_d[t * P:(t + 1) * P, :], o2[:])

            # ---- phase B: gather + segment-sum + transform ----
            for g in range(NG if debug_stop != "A" else 0):
                gx = gxpool.tile([P, NB * (Q // 16)], dt.int16, tag="gidx")
                nc.sync.dma_start(
                    gx[:], gidx_d[:, g * NB * (Q // 16):
                                  (g + 1) * NB * (Q // 16)])
                Gt = []
                for b in range(NB):
                    Gb = gpool.tile([P, GT * Cb, IN2], dt.bfloat16,
                                    tag=f"G{b}")
                    nc.gpsimd.dma_gather(
                        Gb[:], y_full[b * BK:(b + 1) * BK, :],
                        gx[:, b * (Q // 16):(b + 1) * (Q // 16)], Q, Q, IN2,
                        single_packet=(Q <= 1024))
                    Gt.append(Gb)
                if debug_stop == "B":
                    ochk = opool.tile([P, IN2], dt.float32, tag="o1")
                    nc.vector.tensor_copy(ochk[:], Gt[0][:, 0, :])
                    nc.sync.dma_start(out_d[g * P:(g + 1) * P, 0:IN2],
                                      ochk[:])
                    continue
                for tt in range(GT):
                    t = g * GT + tt
                    pagg = ps_agg.tile([P, IN2], dt.float32, tag="pagg")
                    nchunk = NB * Cb
                    ci = 0
                    for b in range(NB):
                        for k in range(Cb):
                            # S = relu(1 - |col - iota|) built on ScalarE --
                            # ACT has its own SBUF ports, so this does not
                            # contend with Q7 SWDGE descriptor generation the
                            # way DVE 2-port-mode ops do.
                            a1 = spool.tile([P, P], dt.bfloat16, tag="a1")
                            cslice = colx_sb[:, (t * NB + b) * Cb + k:
                                             (t * NB + b) * Cb + k + 1]
                            nc.scalar.activation(a1[:], iota_sb[:], Act.Abs,
                                                 bias=cslice, scale=-1.0)
                            S = spool.tile([P, P], dt.bfloat16, tag="S")
                            nc.scalar.activation(S[:], a1[:], Act.Relu,
                                                 bias=1.0, scale=-1.0)
                            nc.tensor.matmul(pagg[:], lhsT=S[:],
                                             rhs=Gt[b][:, tt * Cb + k, :],
                                             start=(ci == 0),
                                             stop=(ci == nchunk - 1))
                            ci += 1
                    aggs = epool.tile([P, IN], dt.float32, tag="aggs")
                    nc.vector.tensor_copy(aggs[:], pagg[:, 0:IN])
                    nc.vector.tensor_tensor(aggs[:], aggs[:],
                                            pagg[:, IN:IN2], Alu.add)
                    agg = epool.tile([P, IN], dt.float32, tag="agg")
                    nc.vector.tensor_scalar(
                        agg[:], aggs[:], dinv_sb[:, t:t + 1], None, Alu.mult)
                    paggT = ps_tr.tile([IN, P], dt.float32, tag="paggT")
                    nc.tensor.transpose(paggT[:], agg[:], ident[:])
                    aggT = epool.tile([IN, P], dt.float32, tag="aggT")
                    nc.vector.tensor_copy(aggT[:], paggT[:])

                    pout = ps_out.tile([P, OUT], dt.float32, tag="pout")
                    nc.tensor.matmul(pout[:], lhsT=aggT[:], rhs=W_sb[:],
                                     start=True, stop=False)
                    nc.tensor.matmul(pout[:], lhsT=xT_sb[:, t * P:(t + 1) * P],
                                     rhs=skipW_sb[:], start=False, stop=True)
                    v = vbuf[:, t * OUT:(t + 1) * OUT]
                    nc.vector.tensor_scalar(
                        v, pout[:], mask_sb[:, t:t + 1], None, Alu.mult)
                    sq = epool.tile([P, OUT], dt.float32, tag="sq")
                    nc.vector.tensor_tensor(sq[:], v, v, Alu.mult)
                    if t == 0:
                        nc.vector.tensor_copy(acc_sum[:], v)
                        nc.vector.tensor_copy(acc_sq[:], sq[:])
                    else:
                        nc.vector.tensor_tensor(acc_sum[:], acc_sum[:], v,
                                                Alu.add)
                        nc.vector.tensor_tensor(acc_sq[:], acc_sq[:], sq[:],
                                                Alu.add)

            # ---- phase C: BN stats allreduce + apply + ReLU ----
            if debug_stop == "C":
                for t in range(T):
                    oc = opool.tile([P, OUT], dt.float32, tag="o2")
                    nc.vector.tensor_copy(oc[:], vbuf[:, t * OUT:(t + 1) * OUT])
                    nc.sync.dma_start(out_d[t * P:(t + 1) * P, :], oc[:])
            if debug_stop == "full":
                pst1 = ps_agg.tile([1, OUT], dt.float32, tag="pagg")
                nc.tensor.matmul(pst1[:], lhsT=ones_col[:], rhs=acc_sum[:],
                                 start=True, stop=True)
                pst2 = ps_tr.tile([1, OUT], dt.float32, tag="paggT")
                nc.tensor.matmul(pst2[:], lhsT=ones_col[:], rhs=acc_sq[:],
                                 start=True, stop=True)
                st_sb = cpool.tile([1, 2 * OUT], dt.float32, tag="st_sb")
                nc.scalar.copy(st_sb[:, 0:OUT], pst1[:])
                nc.scalar.copy(st_sb[:, OUT:2 * OUT], pst2[:])
                nc.sync.dma_start(st_local[:, :], st_sb[:])
                nc.gpsimd.collective_compute(
                    "AllReduce", Alu.add, replica_groups=rg,
                    ins=[st_local.ap().opt()], outs=[st_global.ap().opt()])
                sg_sb = cpool.tile([1, 2 * OUT], dt.float32, tag="sg_sb")
                nc.sync.dma_start(sg_sb[:], st_global[:, :])

                inv_n = 1.0 / float(N)
                mean_sb = cpool.tile([1, OUT], dt.float32, tag="mean_sb")
                nc.vector.tensor_scalar(mean_sb[:], sg_sb[:, 0:OUT], inv_n, None,
                                        Alu.mult)
                var_sb = cpool.tile([1, OUT], dt.float32, tag="var_sb")
                nc.vector.tensor_scalar(var_sb[:], sg_sb[:, OUT:2 * OUT], inv_n,
                                        None, Alu.mult)
                msq = cpool.tile([1, OUT], dt.float32, tag="msq")
                nc.vector.tensor_tensor(msq[:], mean_sb[:], mean_sb[:], Alu.mult)
                nc.vector.tensor_tensor(var_sb[:], var_sb[:], msq[:], Alu.subtract)
                nc.vector.tensor_scalar(var_sb[:], var_sb[:], BN_EPS, None, Alu.add)
                rvar = cpool.tile([1, OUT], dt.float32, tag="rvar")
                nc.vector.reciprocal(rvar[:], var_sb[:])
                rstd = cpool.tile([1, OUT], dt.float32, tag="rstd")
                nc.scalar.activation(rstd[:], rvar[:], Act.Sqrt)

                ab_sb = cpool.tile([1, 2 * OUT], dt.float32, tag="ab_sb")
                nc.vector.tensor_tensor(ab_sb[:, 0:OUT], gamma_sb[:], rstd[:],
                                        Alu.mult)
                ma = cpool.tile([1, OUT], dt.float32, tag="ma")
                nc.vector.tensor_tensor(ma[:], mean_sb[:], ab_sb[:, 0:OUT],
                                        Alu.mult)
                nc.vector.tensor_tensor(ab_sb[:, OUT:2 * OUT], beta_sb[:], ma[:],
                                        Alu.subtract)

                prep = ps_out.tile([P, 2 * OUT], dt.float32, tag="prep")
                nc.tensor.matmul(prep[:], lhsT=ones_row[:], rhs=ab_sb[:],
                                 start=True, stop=True)
                a_rep = cpool.tile([P, OUT], dt.float32, tag="a_rep")
                nc.scalar.copy(a_rep[:], prep[:, 0:OUT])
                b_rep = cpool.tile([P, OUT], dt.float32, tag="b_rep")
                nc.scalar.copy(b_rep[:], prep[:, OUT:2 * OUT])

                for t in range(T):
                    v = vbuf[:, t * OUT:(t + 1) * OUT]
                    o1 = opool.tile([P, OUT], dt.float32, tag="o1")
                    nc.vector.tensor_tensor(o1[:], v, a_rep[:], Alu.mult)
                    nc.vector.tensor_tensor(o1[:], o1[:], b_rep[:], Alu.add)
                    o2 = opool.tile([P, OUT], dt.float32, tag="o2")
                    nc.scalar.activation(o2[:], o1[:], Act.Relu)
                    nc.sync.dma_start(out_d[t * P:(t + 1) * P, :], o2[:])

    nc.compile()
    return nc


def _run(nc, in_maps, M, trace=False):
    from concourse import bass_utils
    res = bass_utils.run_bass_kernel_spmd(
        nc, in_maps, core_ids=list(range(M)), trace=trace)
    return res


def kernel(x, edge_index, W, bias, skip_W, gamma, beta, _trace=False,
           _return_results=False):
    x = np.asarray(x, dtype=np.float32)
    edge_index = np.asarray(edge_index, dtype=np.int32)
    M = 8
    N, IN = x.shape
    OUT = np.asarray(W).shape[1]
    SH = N // M
    T = -(-SH // P)
    GT = 2 if T % 2 == 0 else 1

    in_maps, Cb, NB, SH, T, SHP, node_pos = _host_prep(
        x, edge_index, W, skip_W, gamma, beta, M, IN, OUT, GT)
    key = (M, N, IN, OUT, T, Cb, NB, GT)
    if key not in _KCACHE:
        _KCACHE[key] = _build(M, N, IN, OUT, T, Cb, NB, GT)
    nc = _KCACHE[key]

    res = _run(nc, in_maps, M, trace=_trace)
    outs = [res.results[m]["out"][node_pos[m * SH:(m + 1) * SH]]
            for m in range(M)]
    full = np.concatenate(outs, axis=0).astype(np.float32)
    if _return_results:
        return full, res
    return full



# revision 7
# speedup vs baseline: 2.1128x; 2.1128x over previous
"""GCN message-passing layer (GCNConv + skip + BatchNorm + ReLU) on 8 TRN2 cores.

v2 strategy (evolved from the baseline in kernel_baseline.py):
  - Nodes sharded across 8 cores (12500 each, padded to 12800 = 100*128).
  - y = dinv*x (bf16 hi/lo) AllGathered into one Shared table y_full
    [102400, 128]. Four OVERLAPPING 32768-row windows of y_full serve as
    gather banks (int16 index reach); ~28% of sources fall in two windows,
    and host prep uses that freedom to balance per-(tile,bank) chunk
    counts across banks AND cores, so the instance-compiled chunk
    structure carries ~2% padding instead of the baseline's ~40%.
  - Window/bank b is gathered on SWDGE queue b -> Q7 core pair (2b,2b+1):
    the 4 gather descriptor-generation streams run CONCURRENTLY on the
    GPSIMD engine (measured 3.7x vs the baseline's single queue).
  - Self loops never enter the edge stream: pagg += I^T @ y_own on PE.
  - Per 128-edge chunk: S = (iota == col) built in ONE DVE op (vs 2
    ScalarE ops in the baseline), PE matmul S^T @ G accumulated in PSUM
    per node tile = segment sum in 64-dim space, then one matmul.
  - out = (dinv*(agg+y)) @ W + x @ skip_W  (bias dropped: BN cancels it),
    BN batch stats accumulated on PE (ones^T @ [v, v^2]) with a
    cross-core AllReduce, BN + ReLU applied on device.
"""

import hashlib
import numpy as np
import ml_dtypes

P = 128
M = 8
NBANK = 4
WROW = 32768                 # rows per gather window (int16 reach)

_BF16 = ml_dtypes.bfloat16

_KCACHE = {}


def _host_prep(x, edge_index, W, skip_W, gamma, beta, IN, OUT, GT):
    """Index/layout preprocessing + sharding. All float math on x stays on
    device; here we only partition edges, balance them over bank windows,
    count degrees and lay out per-core arrays."""
    N = x.shape[0]
    SH = N // M
    T = -(-SH // P)
    SHP = T * P
    NFP = M * SHP
    # window starts: cover [0, NFP) with NBANK windows of WROW rows
    wstart = np.round(np.linspace(0, NFP - WROW, NBANK)).astype(np.int64)

    row = edge_index[0].astype(np.int64)
    col = edge_index[1].astype(np.int64)

    deg = np.bincount(col, minlength=N).astype(np.float32) + 1.0  # self loops

    # Degree-balanced node->(tile,slot) assignment per core: snake round-robin
    # over tiles by descending degree equalizes per-tile edge counts.
    node_pos = np.empty(N, dtype=np.int64)
    for m in range(M):
        dg = deg[m * SH:(m + 1) * SH]
        order_n = np.argsort(-dg, kind="stable")
        ranks = np.empty(SH, dtype=np.int64)
        ranks[order_n] = np.arange(SH)
        rounds = ranks // T
        tpos = ranks % T
        tile_of = np.where(rounds % 2 == 0, tpos, T - 1 - tpos)
        node_pos[m * SH:(m + 1) * SH] = tile_of * P + rounds

    # per-edge coordinates
    tcore = col // SH
    tpos = node_pos[col]
    ttile = tpos // P
    tslot = tpos % P
    # y_full rows are partition-major: gr = (core*128 + slot)*T + tile,
    # matching the [P, T, IN2] SBUF layout AllGathered without reshuffling.
    spos = node_pos[row]
    grow = ((row // SH) * P + spos % P) * T + spos // P

    # candidate windows (1 or 2 per edge; windows overlap pairwise)
    hi = np.searchsorted(wstart, grow, side="right") - 1     # last start <= g
    lo_ok = (hi > 0) & (grow < wstart[np.maximum(hi - 1, 0)] + WROW)

    NG = -(-T // GT)

    # --- assign each edge to a window, balancing per (core,tile) ---
    bank = hi.copy()
    key_mt = tcore * T + ttile
    order0 = np.argsort(key_mt, kind="stable")
    bounds = np.searchsorted(key_mt[order0], np.arange(M * T + 1))
    for i in range(M * T):
        e = order0[bounds[i]:bounds[i + 1]]
        if e.size == 0:
            continue
        cb = np.bincount(hi[e], minlength=NBANK).astype(np.int64)
        flex = lo_ok[e]
        fcnt = np.bincount(hi[e][flex], minlength=NBANK)  # movable b -> b-1
        tot = e.size
        # pack banks 3..1 to a multiple-of-128 capacity (minimizes
        # sum of ceil(n_b/128)); slack accumulates in bank 0
        cap = ((tot // NBANK) // P + 1) * P
        mv = np.zeros(NBANK, dtype=np.int64)
        for b in range(NBANK - 1, 0, -1):
            excess = cb[b] - cap
            mv[b] = np.clip(excess, 0, fcnt[b])
            cb[b] -= mv[b]
            cb[b - 1] += mv[b]
        for b in range(1, NBANK):
            if mv[b] > 0:
                cand = e[(hi[e] == b) & lo_ok[e]][:mv[b]]
                bank[cand] = b - 1
    srow = grow - wstart[bank]
    assert (srow >= 0).all() and (srow < WROW).all()

    # --- per-core chunk structure; program uses max over cores ---
    cnts = np.zeros((M, T, NBANK), dtype=np.int64)
    np.add.at(cnts.reshape(-1), (tcore * T + ttile) * NBANK + bank, 1)
    Cb = -(-cnts.max(axis=0) // P)               # [T, NBANK] program chunks
    coff = np.zeros((T, NBANK), dtype=np.int64)
    np.cumsum(Cb.reshape(-1)[:-1], out=coff.reshape(-1)[1:])
    NCH = int(Cb.sum())
    q_call = np.zeros((NG, NBANK), dtype=np.int64)
    for b in range(NBANK):
        for g in range(NG):
            q_call[g, b] = Cb[g * GT:min((g + 1) * GT, T), b].sum() * P
    QMAX = q_call.max(axis=0)
    QMAX = np.maximum(((QMAX + P - 1) // P) * P, P)

    in_maps = []
    for m in range(M):
        e = np.nonzero(tcore == m)[0]
        key = ttile[e] * NBANK + bank[e]
        order = np.argsort(key, kind="stable")
        e = e[order]
        cnt = cnts[m].reshape(-1)
        starts = np.zeros(T * NBANK + 1, dtype=np.int64)
        np.cumsum(cnt, out=starts[1:])
        pos_in_run = np.arange(e.size) - starts[:-1].repeat(cnt)
        ch_of_edge = (coff.reshape(-1).repeat(cnt)[np.arange(e.size)]
                      + pos_in_run // P)
        lane = pos_in_run % P

        colx = np.full((P, NCH), -1.0, dtype=np.float32)
        colx[lane, ch_of_edge] = tslot[e].astype(np.float32)
        gsrc = np.zeros((P, NCH), dtype=np.int16)
        gsrc[lane, ch_of_edge] = srow[e].astype(np.int16)

        gidx_b = []
        for b in range(NBANK):
            blocks = []
            for g in range(NG):
                cols = []
                for t in range(g * GT, min((g + 1) * GT, T)):
                    c0 = coff[t, b]
                    cols.extend(range(c0, c0 + Cb[t, b]))
                q = len(cols) * P
                fl = (gsrc[:, cols].T.reshape(-1) if cols
                      else np.zeros(0, dtype=np.int16))
                fl = np.concatenate(
                    [fl, np.zeros(int(QMAX[b]) - q, dtype=np.int16)])
                blocks.append(
                    np.tile(fl.reshape(int(QMAX[b]) // 16, 16).T, (8, 1)))
            gidx_b.append(np.concatenate(blocks, axis=1).astype(np.int16))

        x_own = np.zeros((SHP, IN), dtype=np.float32)
        x_own[node_pos[m * SH:(m + 1) * SH]] = x[m * SH:(m + 1) * SH]
        deg_own = np.ones(SHP, dtype=np.float32)
        deg_own[node_pos[m * SH:(m + 1) * SH]] = deg[m * SH:(m + 1) * SH]

        im = {
            "xtl": np.ascontiguousarray(
                x_own.reshape(T, P, IN).transpose(1, 0, 2).reshape(P, T * IN)),
            "xT": np.ascontiguousarray(x_own.T),
            "deg": np.ascontiguousarray(deg_own.reshape(T, P).T),
            "colx": np.ascontiguousarray(colx),
            "iota": np.ascontiguousarray(
                np.tile(np.arange(P, dtype=np.float32), (P, 1)).astype(_BF16)),
            "identb": np.ascontiguousarray(
                np.eye(P, dtype=np.float32).astype(_BF16)),
            "ident32": np.ascontiguousarray(np.eye(P, dtype=np.float32)),
            "W": np.ascontiguousarray(W.astype(np.float32)),
            "skipW": np.ascontiguousarray(skip_W.astype(np.float32)),
            "gamma": np.ascontiguousarray(
                gamma.astype(np.float32).reshape(1, OUT)),
            "beta": np.ascontiguousarray(
                beta.astype(np.float32).reshape(1, OUT)),
        }
        for b in range(NBANK):
            im[f"gidx{b}"] = np.ascontiguousarray(gidx_b[b])
        in_maps.append(im)

    return in_maps, Cb, wstart, QMAX, T, node_pos


def _build(N, IN, OUT, T, GT, QMAX, Cb, wstart):
    """Instance-compiled Bass/Tile kernel (chunk structure baked in)."""
    from concourse import bacc, mybir, tile

    dt = mybir.dt
    Alu = mybir.AluOpType
    Act = mybir.ActivationFunctionType

    SHP = T * P
    NFP = M * SHP
    IN2 = 2 * IN
    BN_EPS = 1e-5
    NG = -(-T // GT)
    NCH = int(Cb.sum())
    coff = np.zeros((T, NBANK), dtype=np.int64)
    np.cumsum(Cb.reshape(-1)[:-1], out=coff.reshape(-1)[1:])

    nc = bacc.Bacc("TRN2", target_bir_lowering=False, debug=False,
                   num_devices=M, num_swdge_queues=NBANK)

    xtl_d = nc.dram_tensor("xtl", [P, T * IN], dt.float32, kind="ExternalInput")
    xT_d = nc.dram_tensor("xT", [IN, SHP], dt.float32, kind="ExternalInput")
    deg_d = nc.dram_tensor("deg", [P, T], dt.float32, kind="ExternalInput")
    colx_d = nc.dram_tensor("colx", [P, NCH], dt.float32, kind="ExternalInput")
    iota_d = nc.dram_tensor("iota", [P, P], dt.bfloat16, kind="ExternalInput")
    identb_d = nc.dram_tensor("identb", [P, P], dt.bfloat16,
                              kind="ExternalInput")
    ident32_d = nc.dram_tensor("ident32", [P, P], dt.float32,
                               kind="ExternalInput")
    W_d = nc.dram_tensor("W", [IN, OUT], dt.float32, kind="ExternalInput")
    skipW_d = nc.dram_tensor("skipW", [IN, OUT], dt.float32,
                             kind="ExternalInput")
    gamma_d = nc.dram_tensor("gamma", [1, OUT], dt.float32,
                             kind="ExternalInput")
    beta_d = nc.dram_tensor("beta", [1, OUT], dt.float32, kind="ExternalInput")
    gidx_d = [nc.dram_tensor(f"gidx{b}", [P, NG * (int(QMAX[b]) // 16)],
                             dt.int16, kind="ExternalInput")
              for b in range(NBANK)]
    out_d = nc.dram_tensor("out", [SHP, OUT], dt.float32, kind="ExternalOutput")

    y_local = nc.dram_tensor("y_local", [P, T * IN2], dt.bfloat16)
    y_full = nc.dram_tensor("y_full", [NFP, IN2], dt.bfloat16,
                            addr_space="Shared")
    st_local = nc.dram_tensor("st_local", [1, 2 * OUT], dt.float32)
    st_global = nc.dram_tensor("st_global", [1, 2 * OUT], dt.float32,
                               addr_space="Shared")

    rg = [list(range(M))]

    with tile.TileContext(nc) as tc:
        with (
            tc.tile_pool(name="const", bufs=1) as cpool,
            tc.tile_pool(name="xload", bufs=3) as xpool,
            tc.tile_pool(name="xtload", bufs=3) as xtpool,
            tc.tile_pool(name="sel", bufs=6) as spool,
            tc.tile_pool(name="evac", bufs=3) as epool,
            tc.tile_pool(name="outt", bufs=3) as opool,
            tc.tile_pool(name="ps_agg", bufs=2, space="PSUM") as ps_agg,
            tc.tile_pool(name="ps_tr", bufs=2, space="PSUM") as ps_tr,
            tc.tile_pool(name="ps_out", bufs=2, space="PSUM") as ps_out,
            tc.tile_pool(name="ps_st", bufs=1, space="PSUM") as ps_st,
            tc.tile_pool(name="gidxp", bufs=2) as gxpool,
            tc.tile_pool(name="gather", bufs=3) as gpool,
        ):
            # ---- constants ----
            W_sb = cpool.tile([IN, OUT], dt.float32, tag="W")
            nc.sync.dma_start(W_sb[:], W_d[:, :])
            skipW_sb = cpool.tile([IN, OUT], dt.float32, tag="skipW")
            nc.sync.dma_start(skipW_sb[:], skipW_d[:, :])
            iota_sb = cpool.tile([P, P], dt.bfloat16, tag="iota")
            nc.sync.dma_start(iota_sb[:], iota_d[:, :])
            identb_sb = cpool.tile([P, P], dt.bfloat16, tag="identb")
            nc.sync.dma_start(identb_sb[:], identb_d[:, :])
            ident32_sb = cpool.tile([P, P], dt.float32, tag="ident32")
            nc.sync.dma_start(ident32_sb[:], ident32_d[:, :])
            deg_sb = cpool.tile([P, T], dt.float32, tag="deg")
            nc.sync.dma_start(deg_sb[:], deg_d[:, :])
            colx_sb = cpool.tile([P, NCH], dt.float32, tag="colx")
            nc.sync.dma_start(colx_sb[:], colx_d[:, :])
            gamma_sb = cpool.tile([1, OUT], dt.float32, tag="gamma")
            nc.sync.dma_start(gamma_sb[:], gamma_d[:, :])
            beta_sb = cpool.tile([1, OUT], dt.float32, tag="beta")
            nc.sync.dma_start(beta_sb[:], beta_d[:, :])
            xtl_sb = cpool.tile([P, T * IN], dt.float32, tag="xtl")
            nc.sync.dma_start(xtl_sb[:], xtl_d[:, :])

            ones_bf = cpool.tile([P, 1], dt.bfloat16, tag="ones_bf")
            nc.vector.memset(ones_bf[:], 1.0)
            ones_row = cpool.tile([1, P], dt.float32, tag="ones_row")
            nc.vector.memset(ones_row[:], 1.0)

            y_own = cpool.tile([P, T, IN2], dt.bfloat16, tag="y_own")
            vbuf = cpool.tile([P, T * OUT], dt.bfloat16, tag="vbuf")

            # dinv = sqrt(1/deg)
            dinv_sb = cpool.tile([P, T], dt.float32, tag="dinv")
            rec_t = cpool.tile([P, T], dt.float32, tag="rec_t")
            nc.vector.reciprocal(rec_t[:], deg_sb[:])
            nc.scalar.activation(dinv_sb[:], rec_t[:], Act.Sqrt)

            # ---- phase A: y = dinv*x as bf16 hi/lo, AllGather ----
            for t in range(T):
                y32 = xpool.tile([P, IN], dt.float32, tag="y32")
                nc.vector.tensor_scalar(
                    y32[:], xtl_sb[:, t * IN:(t + 1) * IN],
                    dinv_sb[:, t:t + 1], None, Alu.mult)
                nc.scalar.copy(y_own[:, t, 0:IN], y32[:])
                nc.vector.tensor_tensor(
                    y_own[:, t, IN:IN2], y32[:], y_own[:, t, 0:IN],
                    Alu.subtract)
            nc.sync.dma_start(y_local[:, :], y_own[:, :, :])
            nc.gpsimd.collective_compute(
                "AllGather", Alu.bypass, replica_groups=rg,
                ins=[y_local.ap().opt()], outs=[y_full.ap().opt()])

            # ---- phase B: gathers, 4 concurrent SWDGE queues ----
            gtiles = {}
            for g in range(NG):
                for b in range(NBANK):
                    qb = int(QMAX[b])
                    gx = gxpool.tile([P, qb // 16], dt.int16, tag=f"gx{b}")
                    nc.sync.dma_start(
                        gx[:],
                        gidx_d[b][:, g * (qb // 16):(g + 1) * (qb // 16)])
                    Gb = gpool.tile([P, qb // P, P], dt.bfloat16, tag=f"G{b}")
                    nc.gpsimd.dma_gather(
                        Gb[:],
                        y_full[int(wstart[b]):int(wstart[b]) + WROW, :],
                        gx[:], qb, qb, IN2,
                        single_packet=False, queue_num=b)
                    gtiles[(g, b)] = Gb

            # ---- main loop: segment-sum + transform per tile ----
            ps1 = ps_st.tile([1, 2 * OUT], dt.float32, tag="st")
            for t in range(T):
                g = t // GT
                nch = int(Cb[t].sum())
                pagg = ps_agg.tile([P, IN2], dt.float32, tag="pagg")
                nc.tensor.matmul(pagg[:], lhsT=identb_sb[:],
                                 rhs=y_own[:, t, :], start=True,
                                 stop=(nch == 0))
                ci = 0
                for b in range(NBANK):
                    base = int(np.sum(Cb[g * GT:t, b]))
                    for k in range(int(Cb[t, b])):
                        c = int(coff[t, b]) + k
                        S = spool.tile([P, P], dt.bfloat16, tag="S")
                        nc.vector.tensor_scalar(
                            S[:], iota_sb[:], colx_sb[:, c:c + 1], None,
                            Alu.is_equal)
                        nc.tensor.matmul(pagg[:], lhsT=S[:],
                                         rhs=gtiles[(g, b)][:, base + k, :],
                                         start=False, stop=(ci == nch - 1))
                        ci += 1

                aggh = epool.tile([P, IN], dt.float32, tag="aggh")
                nc.scalar.copy(aggh[:], pagg[:, 0:IN])
                aggs = epool.tile([P, IN], dt.float32, tag="aggs")
                nc.vector.tensor_tensor(aggs[:], aggh[:],
                                        pagg[:, IN:IN2], Alu.add)
                agg = epool.tile([P, IN], dt.float32, tag="agg")
                nc.vector.tensor_scalar(
                    agg[:], aggs[:], dinv_sb[:, t:t + 1], None, Alu.mult)
                paggT = ps_tr.tile([IN, P], dt.float32, tag="paggT")
                nc.tensor.transpose(paggT[:], agg[:], ident32_sb[:])
                aggT = epool.tile([IN, P], dt.float32, tag="aggT")
                nc.scalar.copy(aggT[:], paggT[:])

                xT_t = xtpool.tile([IN, P], dt.float32, tag="xT_t")
                nc.sync.dma_start(xT_t[:], xT_d[:, t * P:(t + 1) * P])
                pout = ps_out.tile([P, OUT], dt.float32, tag="pout")
                nc.tensor.matmul(pout[:], lhsT=aggT[:], rhs=W_sb[:],
                                 start=True, stop=False)
                nc.tensor.matmul(pout[:], lhsT=xT_t[:], rhs=skipW_sb[:],
                                 start=False, stop=True)

                v = vbuf[:, t * OUT:(t + 1) * OUT]
                nc.scalar.copy(v, pout[:])
                sq = epool.tile([P, OUT], dt.bfloat16, tag="sq")
                nc.vector.tensor_tensor(sq[:], v, v, Alu.mult)
                nc.tensor.matmul(ps1[:, 0:OUT], lhsT=ones_bf[:], rhs=v,
                                 start=(t == 0), stop=(t == T - 1))
                nc.tensor.matmul(ps1[:, OUT:2 * OUT], lhsT=ones_bf[:],
                                 rhs=sq[:], start=(t == 0), stop=(t == T - 1))

            # ---- phase C: BN stats allreduce + apply + ReLU ----
            st_sb = cpool.tile([1, 2 * OUT], dt.float32, tag="st_sb")
            nc.scalar.copy(st_sb[:], ps1[:])
            nc.sync.dma_start(st_local[:, :], st_sb[:])
            nc.gpsimd.collective_compute(
                "AllReduce", Alu.add, replica_groups=rg,
                ins=[st_local.ap().opt()], outs=[st_global.ap().opt()])
            sg_sb = cpool.tile([1, 2 * OUT], dt.float32, tag="sg_sb")
            nc.sync.dma_start(sg_sb[:], st_global[:, :])

            inv_n = 1.0 / float(N)
            mean_sb = cpool.tile([1, OUT], dt.float32, tag="mean_sb")
            nc.vector.tensor_scalar(mean_sb[:], sg_sb[:, 0:OUT], inv_n, None,
                                    Alu.mult)
            var_sb = cpool.tile([1, OUT], dt.float32, tag="var_sb")
            nc.vector.tensor_scalar(var_sb[:], sg_sb[:, OUT:2 * OUT], inv_n,
                                    None, Alu.mult)
            msq = cpool.tile([1, OUT], dt.float32, tag="msq")
            nc.vector.tensor_tensor(msq[:], mean_sb[:], mean_sb[:], Alu.mult)
            nc.vector.tensor_tensor(var_sb[:], var_sb[:], msq[:], Alu.subtract)
            nc.vector.tensor_scalar(var_sb[:], var_sb[:], BN_EPS, None, Alu.add)
            rvar = cpool.tile([1, OUT], dt.float32, tag="rvar")
            nc.vector.reciprocal(rvar[:], var_sb[:])
            rstd = cpool.tile([1, OUT], dt.float32, tag="rstd")
            nc.scalar.activation(rstd[:], rvar[:], Act.Sqrt)

            ab_sb = cpool.tile([1, 2 * OUT], dt.float32, tag="ab_sb")
            nc.vector.tensor_tensor(ab_sb[:, 0:OUT], gamma_sb[:], rstd[:],
                                    Alu.mult)
            ma = cpool.tile([1, OUT], dt.float32, tag="ma")
            nc.vector.tensor_tensor(ma[:], mean_sb[:], ab_sb[:, 0:OUT],
                                    Alu.mult)
            nc.vector.tensor_tensor(ab_sb[:, OUT:2 * OUT], beta_sb[:], ma[:],
                                    Alu.subtract)

            prep = ps_out.tile([P, 2 * OUT], dt.float32, tag="pout")
            nc.tensor.matmul(prep[:], lhsT=ones_row[:], rhs=ab_sb[:],
                             start=True, stop=True)
            a_rep = cpool.tile([P, OUT], dt.float32, tag="a_rep")
            nc.scalar.copy(a_rep[:], prep[:, 0:OUT])
            b_rep = cpool.tile([P, OUT], dt.float32, tag="b_rep")
            nc.scalar.copy(b_rep[:], prep[:, OUT:2 * OUT])

            for t in range(T):
                v = vbuf[:, t * OUT:(t + 1) * OUT]
                o1 = opool.tile([P, OUT], dt.float32, tag="o1")
                nc.vector.tensor_tensor(o1[:], v, a_rep[:], Alu.mult)
                nc.vector.tensor_tensor(o1[:], o1[:], b_rep[:], Alu.add)
                o2 = opool.tile([P, OUT], dt.float32, tag="o2")
                nc.scalar.activation(o2[:], o1[:], Act.Relu)
                nc.sync.dma_start(out_d[t * P:(t + 1) * P, :], o2[:])

    nc.compile()
    return nc


def _run(nc, in_maps, trace=False):
    from concourse import bass_utils
    return bass_utils.run_bass_kernel_spmd(
        nc, in_maps, core_ids=list(range(M)), trace=trace)


def kernel(x, edge_index, W, bias, skip_W, gamma, beta, _trace=False,
           _return_results=False):
    x = np.asarray(x, dtype=np.float32)
    edge_index = np.asarray(edge_index, dtype=np.int32)
    N, IN = x.shape
    OUT = np.asarray(W).shape[1]
    GT = 5

    in_maps, Cb, wstart, QMAX, T, node_pos = _host_prep(
        x, edge_index, W, skip_W, gamma, beta, IN, OUT, GT)

    h = hashlib.sha256()
    h.update(Cb.astype(np.int32).tobytes())
    h.update(QMAX.astype(np.int64).tobytes())
    h.update(wstart.astype(np.int64).tobytes())
    key = (N, IN, OUT, T, GT, h.hexdigest())
    if key not in _KCACHE:
        _KCACHE[key] = _build(N, IN, OUT, T, GT, QMAX, Cb, wstart)
    nc = _KCACHE[key]

    res = _run(nc, in_maps, trace=_trace)
    SH = N // M
    outs = [res.results[m]["out"][node_pos[m * SH:(m + 1) * SH]]
            for m in range(M)]
    full = np.concatenate(outs, axis=0).astype(np.float32)
    if _return_results:
        return full, res
    return full


# revision 8
# speedup vs baseline: 3.0360x; 1.4369x over previous
"""GCN message-passing layer (GCNConv + skip + BatchNorm + ReLU) on 8 TRN2 cores.

v2 strategy (evolved from the baseline in kernel_baseline.py):
  - Nodes sharded across 8 cores (12500 each, padded to 12800 = 100*128).
  - y = dinv*x (bf16 hi/lo) AllGathered into one Shared table y_full
    [102400, 128]. Four OVERLAPPING 32768-row windows of y_full serve as
    gather banks (int16 index reach); ~28% of sources fall in two windows,
    and host prep uses that freedom to balance per-(tile,bank) chunk
    counts across banks AND cores, so the instance-compiled chunk
    structure carries ~2% padding instead of the baseline's ~40%.
  - Window/bank b is gathered on SWDGE queue b -> Q7 core pair (2b,2b+1):
    the 4 gather descriptor-generation streams run CONCURRENTLY on the
    GPSIMD engine (measured 3.7x vs the baseline's single queue).
  - Self loops never enter the edge stream: pagg += I^T @ y_own on PE.
  - Per 128-edge chunk: S = (iota == col) built in ONE DVE op (vs 2
    ScalarE ops in the baseline), PE matmul S^T @ G accumulated in PSUM
    per node tile = segment sum in 64-dim space, then one matmul.
  - out = (dinv*(agg+y)) @ W + x @ skip_W  (bias dropped: BN cancels it),
    BN batch stats accumulated on PE (ones^T @ [v, v^2]) with a
    cross-core AllReduce, BN + ReLU applied on device.
"""

import hashlib
import numpy as np
import ml_dtypes

P = 128
M = 8
NBANK = 4
WROW = 32768                 # rows per gather window (int16 reach)

_BF16 = ml_dtypes.bfloat16

_KCACHE = {}


def _host_prep(x, edge_index, W, skip_W, gamma, beta, IN, OUT, GT):
    """Index/layout preprocessing + sharding. All float math on x stays on
    device; here we only partition edges, balance them over bank windows,
    count degrees and lay out per-core arrays."""
    N = x.shape[0]
    SH = N // M
    T = -(-SH // P)
    SHP = T * P
    NFP = M * SHP
    # window starts: cover [0, NFP) with NBANK windows of WROW rows
    wstart = np.round(np.linspace(0, NFP - WROW, NBANK)).astype(np.int64)

    row = edge_index[0].astype(np.int64)
    col = edge_index[1].astype(np.int64)

    deg = np.bincount(col, minlength=N).astype(np.float32) + 1.0  # self loops

    # Degree-balanced node->(tile,slot) assignment per core: snake round-robin
    # over tiles by descending degree equalizes per-tile edge counts.
    node_pos = np.empty(N, dtype=np.int64)
    for m in range(M):
        dg = deg[m * SH:(m + 1) * SH]
        order_n = np.argsort(-dg, kind="stable")
        ranks = np.empty(SH, dtype=np.int64)
        ranks[order_n] = np.arange(SH)
        rounds = ranks // T
        tpos = ranks % T
        tile_of = np.where(rounds % 2 == 0, tpos, T - 1 - tpos)
        node_pos[m * SH:(m + 1) * SH] = tile_of * P + rounds

    # per-edge coordinates
    tcore = col // SH
    tpos = node_pos[col]
    ttile = tpos // P
    tslot = tpos % P
    # y_full rows are partition-major: gr = (core*128 + slot)*T + tile,
    # matching the [P, T, IN2] SBUF layout AllGathered without reshuffling.
    spos = node_pos[row]
    grow = ((row // SH) * P + spos % P) * T + spos // P

    # candidate windows (1 or 2 per edge; windows overlap pairwise)
    hi = np.searchsorted(wstart, grow, side="right") - 1     # last start <= g
    lo_ok = (hi > 0) & (grow < wstart[np.maximum(hi - 1, 0)] + WROW)

    NG = -(-T // GT)

    # --- assign each edge to a window, balancing per (core,tile) ---
    bank = hi.copy()
    key_mt = tcore * T + ttile
    order0 = np.argsort(key_mt, kind="stable")
    bounds = np.searchsorted(key_mt[order0], np.arange(M * T + 1))
    for i in range(M * T):
        e = order0[bounds[i]:bounds[i + 1]]
        if e.size == 0:
            continue
        cb = np.bincount(hi[e], minlength=NBANK).astype(np.int64)
        flex = lo_ok[e]
        fcnt = np.bincount(hi[e][flex], minlength=NBANK)  # movable b -> b-1
        tot = e.size
        # pack banks 3..1 to a multiple-of-128 capacity (minimizes
        # sum of ceil(n_b/128)); slack accumulates in bank 0
        cap = ((tot // NBANK) // P + 1) * P
        mv = np.zeros(NBANK, dtype=np.int64)
        for b in range(NBANK - 1, 0, -1):
            excess = cb[b] - cap
            mv[b] = np.clip(excess, 0, fcnt[b])
            cb[b] -= mv[b]
            cb[b - 1] += mv[b]
        for b in range(1, NBANK):
            if mv[b] > 0:
                cand = e[(hi[e] == b) & lo_ok[e]][:mv[b]]
                bank[cand] = b - 1
    srow = grow - wstart[bank]
    assert (srow >= 0).all() and (srow < WROW).all()

    # --- per-core chunk structure; program uses max over cores ---
    cnts = np.zeros((M, T, NBANK), dtype=np.int64)
    np.add.at(cnts.reshape(-1), (tcore * T + ttile) * NBANK + bank, 1)
    Cb = -(-cnts.max(axis=0) // P)               # [T, NBANK] program chunks
    coff = np.zeros((T, NBANK), dtype=np.int64)
    np.cumsum(Cb.reshape(-1)[:-1], out=coff.reshape(-1)[1:])
    NCH = int(Cb.sum())
    CBMAX = int(Cb.sum(axis=1).max())
    q_call = np.zeros((NG, NBANK), dtype=np.int64)
    for b in range(NBANK):
        for g in range(NG):
            q_call[g, b] = Cb[g * GT:min((g + 1) * GT, T), b].sum() * P
    QMAX = q_call.max(axis=0)
    QMAX = np.maximum(((QMAX + P - 1) // P) * P, P)

    in_maps = []
    for m in range(M):
        e = np.nonzero(tcore == m)[0]
        key = ttile[e] * NBANK + bank[e]
        order = np.argsort(key, kind="stable")
        e = e[order]
        cnt = cnts[m].reshape(-1)
        starts = np.zeros(T * NBANK + 1, dtype=np.int64)
        np.cumsum(cnt, out=starts[1:])
        pos_in_run = np.arange(e.size) - starts[:-1].repeat(cnt)
        ch_of_edge = (coff.reshape(-1).repeat(cnt)[np.arange(e.size)]
                      + pos_in_run // P)
        lane = pos_in_run % P

        colx = np.full((P, NCH), -1.0, dtype=np.float32)
        colx[lane, ch_of_edge] = tslot[e].astype(np.float32)
        colx = colx.astype(_BF16)
        gsrc = np.zeros((P, NCH), dtype=np.int16)
        gsrc[lane, ch_of_edge] = srow[e].astype(np.int16)

        gidx_b = []
        for b in range(NBANK):
            blocks = []
            for g in range(NG):
                cols = []
                for t in range(g * GT, min((g + 1) * GT, T)):
                    c0 = coff[t, b]
                    cols.extend(range(c0, c0 + Cb[t, b]))
                q = len(cols) * P
                fl = (gsrc[:, cols].T.reshape(-1) if cols
                      else np.zeros(0, dtype=np.int16))
                fl = np.concatenate(
                    [fl, np.zeros(int(QMAX[b]) - q, dtype=np.int16)])
                blocks.append(
                    np.tile(fl.reshape(int(QMAX[b]) // 16, 16).T, (8, 1)))
            gidx_b.append(np.concatenate(blocks, axis=1).astype(np.int16))

        x_own = np.zeros((SHP, IN), dtype=np.float32)
        x_own[node_pos[m * SH:(m + 1) * SH]] = x[m * SH:(m + 1) * SH]
        deg_own = np.ones(SHP, dtype=np.float32)
        deg_own[node_pos[m * SH:(m + 1) * SH]] = deg[m * SH:(m + 1) * SH]

        im = {
            "xtl": np.ascontiguousarray(
                x_own.reshape(T, P, IN).transpose(1, 0, 2).reshape(P, T * IN)),
            "xT": np.ascontiguousarray(x_own.T),
            "deg": np.ascontiguousarray(deg_own.reshape(T, P).T),
            "colx": np.ascontiguousarray(colx),
            "iotaw": np.ascontiguousarray(
                np.tile(np.arange(P, dtype=np.float32),
                        (P, CBMAX)).astype(_BF16)),
            "identb": np.ascontiguousarray(
                np.eye(P, dtype=np.float32).astype(_BF16)),
            "ident32": np.ascontiguousarray(np.eye(P, dtype=np.float32)),
            "W": np.ascontiguousarray(W.astype(np.float32)),
            "skipW": np.ascontiguousarray(skip_W.astype(np.float32)),
            "gamma": np.ascontiguousarray(
                gamma.astype(np.float32).reshape(1, OUT)),
            "beta": np.ascontiguousarray(
                beta.astype(np.float32).reshape(1, OUT)),
        }
        for b in range(NBANK):
            im[f"gidx{b}"] = np.ascontiguousarray(gidx_b[b])
        in_maps.append(im)

    return in_maps, Cb, wstart, QMAX, T, node_pos, CBMAX


def _build(N, IN, OUT, T, GT, QMAX, Cb, wstart, CBMAX):
    """Instance-compiled Bass/Tile kernel (chunk structure baked in)."""
    from concourse import bacc, mybir, tile

    dt = mybir.dt
    Alu = mybir.AluOpType
    Act = mybir.ActivationFunctionType

    SHP = T * P
    NFP = M * SHP
    IN2 = 2 * IN
    BN_EPS = 1e-5
    NG = -(-T // GT)
    NCH = int(Cb.sum())
    coff = np.zeros((T, NBANK), dtype=np.int64)
    np.cumsum(Cb.reshape(-1)[:-1], out=coff.reshape(-1)[1:])

    nc = bacc.Bacc("TRN2", target_bir_lowering=False, debug=False,
                   num_devices=M, num_swdge_queues=NBANK)

    xtl_d = nc.dram_tensor("xtl", [P, T * IN], dt.float32, kind="ExternalInput")
    xT_d = nc.dram_tensor("xT", [IN, SHP], dt.float32, kind="ExternalInput")
    deg_d = nc.dram_tensor("deg", [P, T], dt.float32, kind="ExternalInput")
    colx_d = nc.dram_tensor("colx", [P, NCH], dt.bfloat16,
                            kind="ExternalInput")
    iotaw_d = nc.dram_tensor("iotaw", [P, CBMAX * P], dt.bfloat16,
                             kind="ExternalInput")
    identb_d = nc.dram_tensor("identb", [P, P], dt.bfloat16,
                              kind="ExternalInput")
    ident32_d = nc.dram_tensor("ident32", [P, P], dt.float32,
                               kind="ExternalInput")
    W_d = nc.dram_tensor("W", [IN, OUT], dt.float32, kind="ExternalInput")
    skipW_d = nc.dram_tensor("skipW", [IN, OUT], dt.float32,
                             kind="ExternalInput")
    gamma_d = nc.dram_tensor("gamma", [1, OUT], dt.float32,
                             kind="ExternalInput")
    beta_d = nc.dram_tensor("beta", [1, OUT], dt.float32, kind="ExternalInput")
    gidx_d = [nc.dram_tensor(f"gidx{b}", [P, NG * (int(QMAX[b]) // 16)],
                             dt.int16, kind="ExternalInput")
              for b in range(NBANK)]
    out_d = nc.dram_tensor("out", [SHP, OUT], dt.float32, kind="ExternalOutput")

    y_local = nc.dram_tensor("y_local", [P, T * IN2], dt.bfloat16)
    y_full = nc.dram_tensor("y_full", [NFP, IN2], dt.bfloat16,
                            addr_space="Shared")
    st_local = nc.dram_tensor("st_local", [1, 2 * OUT], dt.float32)
    st_global = nc.dram_tensor("st_global", [1, 2 * OUT], dt.float32,
                               addr_space="Shared")

    rg = [list(range(M))]

    with tile.TileContext(nc) as tc:
        with (
            tc.tile_pool(name="const", bufs=1) as cpool,
            tc.tile_pool(name="xload", bufs=3) as xpool,
            tc.tile_pool(name="xtload", bufs=3) as xtpool,
            tc.tile_pool(name="sel", bufs=3) as spool,
            tc.tile_pool(name="evac", bufs=3) as epool,
            tc.tile_pool(name="outt", bufs=3) as opool,
            tc.tile_pool(name="ps_agg", bufs=2, space="PSUM") as ps_agg,
            tc.tile_pool(name="ps_tr", bufs=2, space="PSUM") as ps_tr,
            tc.tile_pool(name="ps_out", bufs=2, space="PSUM") as ps_out,
            tc.tile_pool(name="ps_st", bufs=1, space="PSUM") as ps_st,
            tc.tile_pool(name="gidxp", bufs=2) as gxpool,
            tc.tile_pool(name="gather", bufs=4) as gpool,
        ):
            # ---- constants ----
            W_sb = cpool.tile([IN, OUT], dt.float32, tag="W")
            nc.sync.dma_start(W_sb[:], W_d[:, :])
            skipW_sb = cpool.tile([IN, OUT], dt.float32, tag="skipW")
            nc.sync.dma_start(skipW_sb[:], skipW_d[:, :])
            iotaw_sb = cpool.tile([P, CBMAX, P], dt.bfloat16, tag="iotaw")
            nc.sync.dma_start(iotaw_sb[:], iotaw_d[:, :])
            identb_sb = cpool.tile([P, P], dt.bfloat16, tag="identb")
            nc.sync.dma_start(identb_sb[:], identb_d[:, :])
            ident32_sb = cpool.tile([P, P], dt.float32, tag="ident32")
            nc.sync.dma_start(ident32_sb[:], ident32_d[:, :])
            deg_sb = cpool.tile([P, T], dt.float32, tag="deg")
            nc.sync.dma_start(deg_sb[:], deg_d[:, :])
            colx_sb = cpool.tile([P, NCH], dt.bfloat16, tag="colx")
            nc.sync.dma_start(colx_sb[:], colx_d[:, :])
            gamma_sb = cpool.tile([1, OUT], dt.float32, tag="gamma")
            nc.sync.dma_start(gamma_sb[:], gamma_d[:, :])
            beta_sb = cpool.tile([1, OUT], dt.float32, tag="beta")
            nc.sync.dma_start(beta_sb[:], beta_d[:, :])

            ones_bf = cpool.tile([P, 1], dt.bfloat16, tag="ones_bf")
            nc.vector.memset(ones_bf[:], 1.0)
            ones_row = cpool.tile([1, P], dt.float32, tag="ones_row")
            nc.vector.memset(ones_row[:], 1.0)

            y_own = cpool.tile([P, T, IN2], dt.bfloat16, tag="y_own")
            vbuf = cpool.tile([P, T * OUT], dt.bfloat16, tag="vbuf")

            # dinv = sqrt(1/deg)
            dinv_sb = cpool.tile([P, T], dt.float32, tag="dinv")
            rec_t = cpool.tile([P, T], dt.float32, tag="rec_t")
            nc.vector.reciprocal(rec_t[:], deg_sb[:])
            nc.scalar.activation(dinv_sb[:], rec_t[:], Act.Sqrt)

            # ---- phase A: y = dinv*x as bf16 hi/lo, AllGather ----
            TS = 25
            for s0 in range(0, T, TS):
                s1 = min(s0 + TS, T)
                xsl = xpool.tile([P, TS * IN], dt.float32, tag="xsl")
                nc.sync.dma_start(xsl[:, 0:(s1 - s0) * IN],
                                  xtl_d[:, s0 * IN:s1 * IN])
                for t in range(s0, s1):
                    y32 = xpool.tile([P, IN], dt.float32, tag="y32")
                    nc.vector.tensor_scalar(
                        y32[:], xsl[:, (t - s0) * IN:(t - s0 + 1) * IN],
                        dinv_sb[:, t:t + 1], None, Alu.mult)
                    nc.scalar.copy(y_own[:, t, 0:IN], y32[:])
                    nc.vector.tensor_tensor(
                        y_own[:, t, IN:IN2], y32[:], y_own[:, t, 0:IN],
                        Alu.subtract)
            nc.sync.dma_start(y_local[:, :], y_own[:, :, :])
            nc.gpsimd.collective_compute(
                "AllGather", Alu.bypass, replica_groups=rg,
                ins=[y_local.ap().opt()], outs=[y_full.ap().opt()])

            # ---- phase B: gathers, 4 concurrent SWDGE queues ----
            gtiles = {}
            for g in range(NG):
                for b in range(NBANK):
                    qb = int(QMAX[b])
                    gx = gxpool.tile([P, qb // 16], dt.int16, tag=f"gx{b}")
                    nc.sync.dma_start(
                        gx[:],
                        gidx_d[b][:, g * (qb // 16):(g + 1) * (qb // 16)])
                    Gb = gpool.tile([P, qb // P, P], dt.bfloat16, tag=f"G{b}")
                    nc.gpsimd.dma_gather(
                        Gb[:],
                        y_full[int(wstart[b]):int(wstart[b]) + WROW, :],
                        gx[:], qb, qb, IN2,
                        single_packet=False, queue_num=b)
                    gtiles[(g, b)] = Gb

            # ---- main loop: segment-sum + transform per tile ----
            ps1 = ps_st.tile([1, 2 * OUT], dt.float32, tag="st")
            for t in range(T):
                g = t // GT
                nch = int(Cb[t].sum())
                pagg = ps_agg.tile([P, IN2], dt.float32, tag="pagg")
                nc.tensor.matmul(pagg[:], lhsT=identb_sb[:],
                                 rhs=y_own[:, t, :], start=True,
                                 stop=(nch == 0))
                if nch > 0:
                    c0 = int(coff[t, 0])
                    S = spool.tile([P, CBMAX, P], dt.bfloat16, tag="S")
                    nc.vector.tensor_tensor(
                        S[:, 0:nch, :], iotaw_sb[:, 0:nch, :],
                        colx_sb[:, c0:c0 + nch].broadcast_to([P, nch, P]),
                        Alu.is_equal)
                ci = 0
                for b in range(NBANK):
                    base = int(np.sum(Cb[g * GT:t, b]))
                    for k in range(int(Cb[t, b])):
                        c = int(coff[t, b]) + k - int(coff[t, 0])
                        nc.tensor.matmul(pagg[:], lhsT=S[:, c, :],
                                         rhs=gtiles[(g, b)][:, base + k, :],
                                         start=False, stop=(ci == nch - 1))
                        ci += 1

                aggh = epool.tile([P, IN], dt.float32, tag="aggh")
                nc.scalar.copy(aggh[:], pagg[:, 0:IN])
                aggs = epool.tile([P, IN], dt.float32, tag="aggs")
                nc.vector.tensor_tensor(aggs[:], aggh[:],
                                        pagg[:, IN:IN2], Alu.add)
                agg = epool.tile([P, IN], dt.float32, tag="agg")
                nc.vector.tensor_scalar(
                    agg[:], aggs[:], dinv_sb[:, t:t + 1], None, Alu.mult)
                paggT = ps_tr.tile([IN, P], dt.float32, tag="paggT")
                nc.tensor.transpose(paggT[:], agg[:], ident32_sb[:])
                aggT = epool.tile([IN, P], dt.float32, tag="aggT")
                nc.scalar.copy(aggT[:], paggT[:])

                xT_t = xtpool.tile([IN, P], dt.float32, tag="xT_t")
                nc.sync.dma_start(xT_t[:], xT_d[:, t * P:(t + 1) * P])
                pout = ps_out.tile([P, OUT], dt.float32, tag="pout")
                nc.tensor.matmul(pout[:], lhsT=aggT[:], rhs=W_sb[:],
                                 start=True, stop=False)
                nc.tensor.matmul(pout[:], lhsT=xT_t[:], rhs=skipW_sb[:],
                                 start=False, stop=True)

                v = vbuf[:, t * OUT:(t + 1) * OUT]
                nc.scalar.copy(v, pout[:])
                sq = epool.tile([P, OUT], dt.bfloat16, tag="sq")
                nc.vector.tensor_tensor(sq[:], v, v, Alu.mult)
                nc.tensor.matmul(ps1[:, 0:OUT], lhsT=ones_bf[:], rhs=v,
                                 start=(t == 0), stop=(t == T - 1))
                nc.tensor.matmul(ps1[:, OUT:2 * OUT], lhsT=ones_bf[:],
                                 rhs=sq[:], start=(t == 0), stop=(t == T - 1))

            # ---- phase C: BN stats allreduce + apply + ReLU ----
            st_sb = cpool.tile([1, 2 * OUT], dt.float32, tag="st_sb")
            nc.scalar.copy(st_sb[:], ps1[:])
            nc.sync.dma_start(st_local[:, :], st_sb[:])
            nc.gpsimd.collective_compute(
                "AllReduce", Alu.add, replica_groups=rg,
                ins=[st_local.ap().opt()], outs=[st_global.ap().opt()])
            sg_sb = cpool.tile([1, 2 * OUT], dt.float32, tag="sg_sb")
            nc.sync.dma_start(sg_sb[:], st_global[:, :])

            inv_n = 1.0 / float(N)
            mean_sb = cpool.tile([1, OUT], dt.float32, tag="mean_sb")
            nc.vector.tensor_scalar(mean_sb[:], sg_sb[:, 0:OUT], inv_n, None,
                                    Alu.mult)
            var_sb = cpool.tile([1, OUT], dt.float32, tag="var_sb")
            nc.vector.tensor_scalar(var_sb[:], sg_sb[:, OUT:2 * OUT], inv_n,
                                    None, Alu.mult)
            msq = cpool.tile([1, OUT], dt.float32, tag="msq")
            nc.vector.tensor_tensor(msq[:], mean_sb[:], mean_sb[:], Alu.mult)
            nc.vector.tensor_tensor(var_sb[:], var_sb[:], msq[:], Alu.subtract)
            nc.vector.tensor_scalar(var_sb[:], var_sb[:], BN_EPS, None, Alu.add)
            rvar = cpool.tile([1, OUT], dt.float32, tag="rvar")
            nc.vector.reciprocal(rvar[:], var_sb[:])
            rstd = cpool.tile([1, OUT], dt.float32, tag="rstd")
            nc.scalar.activation(rstd[:], rvar[:], Act.Sqrt)

            ab_sb = cpool.tile([1, 2 * OUT], dt.float32, tag="ab_sb")
            nc.vector.tensor_tensor(ab_sb[:, 0:OUT], gamma_sb[:], rstd[:],
                                    Alu.mult)
            ma = cpool.tile([1, OUT], dt.float32, tag="ma")
            nc.vector.tensor_tensor(ma[:], mean_sb[:], ab_sb[:, 0:OUT],
                                    Alu.mult)
            nc.vector.tensor_tensor(ab_sb[:, OUT:2 * OUT], beta_sb[:], ma[:],
                                    Alu.subtract)

            prep = ps_out.tile([P, 2 * OUT], dt.float32, tag="pout")
            nc.tensor.matmul(prep[:], lhsT=ones_row[:], rhs=ab_sb[:],
                             start=True, stop=True)
            a_rep = cpool.tile([P, OUT], dt.float32, tag="a_rep")
            nc.scalar.copy(a_rep[:], prep[:, 0:OUT])
            b_rep = cpool.tile([P, OUT], dt.float32, tag="b_rep")
            nc.scalar.copy(b_rep[:], prep[:, OUT:2 * OUT])

            for t in range(T):
                v = vbuf[:, t * OUT:(t + 1) * OUT]
                o1 = opool.tile([P, OUT], dt.float32, tag="o1")
                nc.vector.tensor_tensor(o1[:], v, a_rep[:], Alu.mult)
                nc.vector.tensor_tensor(o1[:], o1[:], b_rep[:], Alu.add)
                o2 = opool.tile([P, OUT], dt.float32, tag="o2")
                nc.scalar.activation(o2[:], o1[:], Act.Relu)
                nc.sync.dma_start(out_d[t * P:(t + 1) * P, :], o2[:])

    nc.compile()
    return nc


def _run(nc, in_maps, trace=False):
    from concourse import bass_utils
    return bass_utils.run_bass_kernel_spmd(
        nc, in_maps, core_ids=list(range(M)), trace=trace)


def kernel(x, edge_index, W, bias, skip_W, gamma, beta, _trace=False,
           _return_results=False):
    x = np.asarray(x, dtype=np.float32)
    edge_index = np.asarray(edge_index, dtype=np.int32)
    N, IN = x.shape
    OUT = np.asarray(W).shape[1]
    GT = 3

    in_maps, Cb, wstart, QMAX, T, node_pos, CBMAX = _host_prep(
        x, edge_index, W, skip_W, gamma, beta, IN, OUT, GT)

    h = hashlib.sha256()
    h.update(Cb.astype(np.int32).tobytes())
    h.update(QMAX.astype(np.int64).tobytes())
    h.update(wstart.astype(np.int64).tobytes())
    key = (N, IN, OUT, T, GT, h.hexdigest())
    if key not in _KCACHE:
        _KCACHE[key] = _build(N, IN, OUT, T, GT, QMAX, Cb, wstart, CBMAX)
    nc = _KCACHE[key]

    res = _run(nc, in_maps, trace=_trace)
    SH = N // M
    outs = [res.results[m]["out"][node_pos[m * SH:(m + 1) * SH]]
            for m in range(M)]
    full = np.concatenate(outs, axis=0).astype(np.float32)
    if _return_results:
        return full, res
    return full
